# revision 10
# baseline (speedup 1.0000x reference)
"""Cross-Mamba selective-scan (2 branches, swapped C) on 8 Trainium2 NeuronCores.

Sharding: core c = 2*b + s handles batch b, stream s (s=0: rgb branch,
s=1: e branch).  Each core runs the full selective scan + layernorm for one
(batch, branch) pair with d_inner = 384 on-chip.

Per-core device pipeline:
  A) PE-transpose u (own stream) and xo (other stream) to d-major xT/xoT.
  B) PE matmuls: delta_raw = MT.T @ xT  (MT = (dtW @ Wxp[:R]).T, host-fused),
     B = WB.T @ xT, C = WC.T @ xoT (C from the *other* stream = cross attn).
     softplus via ACT: Ln(Exp(x + dtb) + 1).
  C) For each state idx n: broadcast B[n]/C[n] across partitions (gpsimd),
     dA = Exp(delta * A[:,n]) (ACT, per-partition scale),
     dBu = delta*u*B (DVE), hardware scan x_l = dA*x + dBu (DVE
     tensor_tensor_scan, in-place), y += xs*C (DVE).
  D) Stats over d via PE ones-matmul, transpose y back to l-major,
     layernorm, DMA out.
"""
import os
import sys

sys.path.insert(0, "/opt/trn_rl_repo")
os.environ.setdefault("MYCRO_LOCAL_CACHE", "1")

from contextlib import ExitStack

import numpy as np

B_, L, DI, N, R = 4, 2048, 384, 16, 12
P = 128
NDB = DI // P      # 3 d-blocks of 128 channels
NLT = L // P       # 16 l-tiles of 128
LC = 512           # l-chunk for matmuls / psum
NLC = L // LC      # 4
EPS = 1e-5

TRACE = False
LAST = {}

_prog = None


def _build():
    import concourse.bacc as bacc
    import concourse.tile as tile
    from concourse import library_config, mybir

    f32 = mybir.dt.float32
    AF = mybir.ActivationFunctionType
    OP = mybir.AluOpType

    nc = bacc.Bacc("TRN2", target_bir_lowering=False, debug=False, num_devices=8)

    def inp(name, shape):
        return nc.dram_tensor(name, list(shape), f32, kind="ExternalInput").ap()

    u_d = inp("u", (L, DI))
    xo_d = inp("xo", (L, DI))
    mt_d = inp("mt", (DI, DI))
    wb_d = inp("wb", (DI, N))
    wc_d = inp("wc", (DI, N))
    ar_d = inp("a_r", (P, NDB * N))
    dtb_d = inp("dtb_r", (P, NDB))
    dr_d = inp("d_r", (P, NDB))
    g_d = inp("g_rep", (P, DI))
    bb_d = inp("b_rep", (P, DI))
    id_d = inp("ident", (P, P))
    on_d = inp("ones", (P, 1))
    y_d = nc.dram_tensor("y", [L, DI], f32, kind="ExternalOutput").ap()

    with tile.TileContext(nc) as tc, ExitStack() as ctx:
        nc.gpsimd.load_library(library_config.attn)

        consts = ctx.enter_context(tc.tile_pool(name="consts", bufs=1))
        big = ctx.enter_context(tc.tile_pool(name="bigp", bufs=8))
        dsp_p = ctx.enter_context(tc.tile_pool(name="dspp", bufs=3))
        du_p = ctx.enter_context(tc.tile_pool(name="dup", bufs=3))
        rep_p = ctx.enter_context(tc.tile_pool(name="repp", bufs=2))
        row_p = ctx.enter_context(tc.tile_pool(name="rowp", bufs=1))
        stage_p = ctx.enter_context(tc.tile_pool(name="stagep", bufs=5))
        etmp_p = ctx.enter_context(tc.tile_pool(name="etmpp", bufs=1))
        bcst_p = ctx.enter_context(tc.tile_pool(name="bcstp", bufs=2))
        ylm_p = ctx.enter_context(tc.tile_pool(name="ylmp", bufs=2))
        dram_p = ctx.enter_context(tc.tile_pool(name="dramp", bufs=1, space="DRAM"))
        ps_big = ctx.enter_context(tc.tile_pool(name="psbig", bufs=4, space="PSUM"))
        ps_sm = ctx.enter_context(tc.tile_pool(name="pssm", bufs=2, space="PSUM"))

        # ---- constants ----
        mt_sb = consts.tile([P, NDB, DI], f32, tag="mt")
        nc.sync.dma_start(mt_sb[:], mt_d.rearrange("(k p) d -> p k d", p=P))
        wb_sb = consts.tile([P, NDB, N], f32, tag="wb")
        nc.sync.dma_start(wb_sb[:], wb_d.rearrange("(k p) n -> p k n", p=P))
        wc_sb = consts.tile([P, NDB, N], f32, tag="wc")
        nc.sync.dma_start(wc_sb[:], wc_d.rearrange("(k p) n -> p k n", p=P))
        ar_sb = consts.tile([P, NDB * N], f32, tag="ar")
        nc.sync.dma_start(ar_sb[:], ar_d[:])
        dtb_sb = consts.tile([P, NDB], f32, tag="dtb")
        nc.sync.dma_start(dtb_sb[:], dtb_d[:])
        dr_sb = consts.tile([P, NDB], f32, tag="dr")
        nc.sync.dma_start(dr_sb[:], dr_d[:])
        g_sb = consts.tile([P, DI], f32, tag="g")
        nc.sync.dma_start(g_sb[:], g_d[:])
        bb_sb = consts.tile([P, DI], f32, tag="bb")
        nc.sync.dma_start(bb_sb[:], bb_d[:])
        ident = consts.tile([P, P], f32, tag="ident")
        nc.sync.dma_start(ident[:], id_d[:])
        ones_sb = consts.tile([P, 1], f32, tag="ones")
        nc.sync.dma_start(ones_sb[:], on_d[:])

        # ---- A: transpose inputs to d-major ----
        def load_transpose(src, dst):
            for lc in range(NLC):
                stages = []
                for q in range(4):
                    st = stage_p.tile([P, DI], f32, tag="stage")
                    lt = lc * 4 + q
                    nc.sync.dma_start(st[:], src[lt * P:(lt + 1) * P, :])
                    stages.append(st)
                for db in range(NDB):
                    ps = ps_big.tile([P, LC], f32, tag="psbig")
                    for q in range(4):
                        nc.tensor.transpose(
                            ps[:, q * P:(q + 1) * P],
                            stages[q][:, db * P:(db + 1) * P],
                            ident[:],
                        )
                    nc.scalar.copy(dst[db][:, lc * LC:(lc + 1) * LC], ps[:])

        xT = [big.tile([P, L], f32, tag="big", name=f"xT{i}") for i in range(NDB)]
        load_transpose(u_d, xT)
        xoT = [big.tile([P, L], f32, tag="big", name=f"xoT{i}") for i in range(NDB)]
        load_transpose(xo_d, xoT)

        # ---- B: projections ----
        b_dram = dram_p.tile([N, L], f32, tag="bdram")
        c_dram = dram_p.tile([N, L], f32, tag="cdram")
        for lc in range(NLC):
            sl = slice(lc * LC, (lc + 1) * LC)
            psb = ps_sm.tile([N, LC], f32, tag="pssm")
            for k in range(NDB):
                nc.tensor.matmul(psb[:], wb_sb[:, k, :], xT[k][:, sl],
                                 start=(k == 0), stop=(k == NDB - 1))
            bst = bcst_p.tile([N, LC], f32, tag="bcst")
            nc.scalar.copy(bst[:], psb[:])
            nc.sync.dma_start(b_dram[:, sl], bst[:])
            psc = ps_sm.tile([N, LC], f32, tag="pssm")
            for k in range(NDB):
                nc.tensor.matmul(psc[:], wc_sb[:, k, :], xoT[k][:, sl],
                                 start=(k == 0), stop=(k == NDB - 1))
            cst = bcst_p.tile([N, LC], f32, tag="bcst")
            nc.scalar.copy(cst[:], psc[:])
            nc.sync.dma_start(c_dram[:, sl], cst[:])

        dsps, dus, yaccs = [], [], []
        for db in range(NDB):
            dsp = dsp_p.tile([P, L], f32, tag="dsp")
            for lc in range(NLC):
                sl = slice(lc * LC, (lc + 1) * LC)
                ps = ps_big.tile([P, LC], f32, tag="psbig")
                for k in range(NDB):
                    nc.tensor.matmul(ps[:], mt_sb[:, k, db * P:(db + 1) * P],
                                     xT[k][:, sl],
                                     start=(k == 0), stop=(k == NDB - 1))
                # softplus(x + dtb) = Ln(Exp(x + dtb) + 1)
                et = etmp_p.tile([P, LC], f32, tag="etmp")
                nc.scalar.activation(et[:], ps[:], AF.Exp,
                                     bias=dtb_sb[:, db:db + 1], scale=1.0)
                nc.scalar.activation(dsp[:, sl], et[:], AF.Ln, bias=1.0, scale=1.0)
            dsps.append(dsp)
            du = du_p.tile([P, L], f32, tag="du")
            nc.vector.tensor_mul(du[:], dsp[:], xT[db][:])
            dus.append(du)
            yacc = big.tile([P, L], f32, tag="big")
            nc.vector.tensor_scalar_mul(yacc[:], xT[db][:], dr_sb[:, db:db + 1])
            yaccs.append(yacc)

        # ---- C: scan over state dim ----
        for n in range(N):
            brow = row_p.tile([1, L], f32, tag="brow")
            nc.sync.dma_start(brow[:], b_dram[n:n + 1, :])
            brep = rep_p.tile([P, L], f32, tag="brep")
            nc.gpsimd.partition_broadcast(brep[:], brow[:])
            crow = row_p.tile([1, L], f32, tag="crow")
            nc.sync.dma_start(crow[:], c_dram[n:n + 1, :])
            crep = rep_p.tile([P, L], f32, tag="crep")
            nc.gpsimd.partition_broadcast(crep[:], crow[:])
            for db in range(NDB):
                dA = big.tile([P, L], f32, tag="big")
                nc.scalar.activation(dA[:], dsps[db][:], AF.Exp,
                                     scale=ar_sb[:, db * N + n: db * N + n + 1])
                dBu = big.tile([P, L], f32, tag="big")
                nc.vector.tensor_mul(dBu[:], dus[db][:], brep[:])
                # in-place scan: dA becomes xs
                nc.vector.tensor_tensor_scan(dA[:], dA[:], dBu[:], 0.0,
                                             OP.mult, OP.add)
                nc.vector.tensor_mul(dA[:], dA[:], crep[:])
                nc.vector.tensor_add(yaccs[db][:], yaccs[db][:], dA[:])

        # ---- D: layernorm stats (over d) via PE, transpose back, normalize ----
        sqs = []
        for db in range(NDB):
            sq = big.tile([P, L], f32, tag="big")
            nc.scalar.activation(sq[:], yaccs[db][:], AF.Square, scale=1.0)
            sqs.append(sq)
        sum_sb = consts.tile([NLT, P], f32, tag="sumsb")
        ssq_sb = consts.tile([NLT, P], f32, tag="ssqsb")
        for lc in range(NLC):
            sl = slice(lc * LC, (lc + 1) * LC)
            pss = ps_sm.tile([1, LC], f32, tag="pssm")
            for k in range(NDB):
                nc.tensor.matmul(pss[:], ones_sb[:], yaccs[k][:, sl],
                                 start=(k == 0), stop=(k == NDB - 1))
            sst = bcst_p.tile([1, LC], f32, tag="bcst")
            nc.scalar.copy(sst[:], pss[:])
            nc.sync.dma_start(sum_sb[lc * 4:(lc + 1) * 4, :],
                              sst[:].rearrange("o (b c) -> (o b) c", b=4))
            psq = ps_sm.tile([1, LC], f32, tag="pssm")
            for k in range(NDB):
                nc.tensor.matmul(psq[:], ones_sb[:], sqs[k][:, sl],
                                 start=(k == 0), stop=(k == NDB - 1))
            qst = bcst_p.tile([1, LC], f32, tag="bcst")
            nc.scalar.copy(qst[:], psq[:])
            nc.sync.dma_start(ssq_sb[lc * 4:(lc + 1) * 4, :],
                              qst[:].rearrange("o (b c) -> (o b) c", b=4))
        # transpose (16,128) stats -> (128,16) per-partition columns
        mean = consts.tile([P, NLT], f32, tag="mean")
        rstd = consts.tile([P, NLT], f32, tag="rstd")
        msq = consts.tile([P, NLT], f32, tag="msq")
        pst = ps_sm.tile([P, NLT], f32, tag="pssm")
        nc.tensor.transpose(pst[:], sum_sb[:], ident[:NLT, :NLT])
        nc.scalar.mul(mean[:], pst[:], 1.0 / DI)
        pst2 = ps_sm.tile([P, NLT], f32, tag="pssm")
        nc.tensor.transpose(pst2[:], ssq_sb[:], ident[:NLT, :NLT])
        nc.scalar.mul(msq[:], pst2[:], 1.0 / DI)
        # var = msq - mean^2 ; rstd = exp(-0.5*ln(var + eps))
        var = consts.tile([P, NLT], f32, tag="var")
        nc.vector.tensor_mul(var[:], mean[:], mean[:])
        nc.vector.tensor_sub(var[:], msq[:], var[:])
        nc.vector.tensor_scalar_add(var[:], var[:], EPS)
        nc.scalar.activation(var[:], var[:], AF.Ln, scale=1.0)
        nc.scalar.activation(rstd[:], var[:], AF.Exp, scale=-0.5)

        for lt in range(NLT):
            psy = ps_big.tile([P, DI], f32, tag="psbig")
            for db in range(NDB):
                nc.tensor.transpose(psy[:, db * P:(db + 1) * P],
                                    yaccs[db][:, lt * P:(lt + 1) * P], ident[:])
            ylm = ylm_p.tile([P, DI], f32, tag="ylm")
            nc.scalar.copy(ylm[:], psy[:])
            nc.vector.tensor_scalar(ylm[:], ylm[:], mean[:, lt:lt + 1],
                                    rstd[:, lt:lt + 1], OP.subtract, OP.mult)
            nc.vector.tensor_mul(ylm[:], ylm[:], g_sb[:])
            nc.vector.tensor_add(ylm[:], ylm[:], bb_sb[:])
            nc.sync.dma_start(y_d[lt * P:(lt + 1) * P, :], ylm[:])

    nc.compile()
    return nc


def _get_prog():
    global _prog
    if _prog is None:
        _prog = _build()
    return _prog


def _host_prep(Wxp, dtW, dtb, A_log, D, g, b, Wxp_other):
    """Per-branch constant tensors (host-side weight fusion / relayout)."""
    Wxp = np.asarray(Wxp, np.float32)
    Wxp_other = np.asarray(Wxp_other, np.float32)
    dtW = np.asarray(dtW, np.float32)
    mt = (dtW @ Wxp[:R]).T.copy()                     # (DI, DI) [d', d]
    wb = Wxp[R:R + N].T.copy()                        # (DI, N)
    wc = Wxp_other[R + N:R + 2 * N].T.copy()          # (DI, N)  cross-C
    A_neg = -np.exp(np.asarray(A_log, np.float32))    # (DI, N)
    a_r = A_neg.reshape(NDB, P, N).transpose(1, 0, 2).reshape(P, NDB * N).copy()
    dtb_r = np.asarray(dtb, np.float32).reshape(NDB, P).T.copy()
    d_r = np.asarray(D, np.float32).reshape(NDB, P).T.copy()
    g_rep = np.broadcast_to(np.asarray(g, np.float32), (P, DI)).copy()
    b_rep = np.broadcast_to(np.asarray(b, np.float32), (P, DI)).copy()
    return dict(mt=mt, wb=wb, wc=wc, a_r=a_r, dtb_r=dtb_r, d_r=d_r,
                g_rep=g_rep, b_rep=b_rep,
                ident=np.eye(P, dtype=np.float32),
                ones=np.ones((P, 1), np.float32))


def kernel(x_rgb, x_e, Wxp1, Wxp2, dtW1, dtW2, dtb1, dtb2,
           A_log1, A_log2, D1, D2, g1, b1, g2, b2):
    from concourse.bass_utils import run_bass_kernel_spmd

    nc = _get_prog()
    x_rgb = np.asarray(x_rgb, np.float32)
    x_e = np.asarray(x_e, np.float32)
    w1 = _host_prep(Wxp1, dtW1, dtb1, A_log1, D1, g1, b1, Wxp2)
    w2 = _host_prep(Wxp2, dtW2, dtb2, A_log2, D2, g2, b2, Wxp1)

    in_maps = []
    for b in range(B_):
        in_maps.append(dict(u=x_rgb[b], xo=x_e[b], **w1))
        in_maps.append(dict(u=x_e[b], xo=x_rgb[b], **w2))

    res = run_bass_kernel_spmd(nc, in_maps, list(range(8)), trace=TRACE)
    LAST["res"] = res
    ys = [res.results[i]["y"] for i in range(8)]
    out_rgb = np.stack([ys[2 * b] for b in range(B_)])
    out_e = np.stack([ys[2 * b + 1] for b in range(B_)])
    return out_rgb, out_e
